# revision 22
# baseline (speedup 1.0000x reference)
"""Cross-attention kernel for Trainium2, 8 NeuronCores.

Sharding (data + head parallel, per the problem's sharding hint):
  core c in 0..7 -> batch b = c // 4, head-pair hp = c % 4.
  Each core computes attention for its batch with 2 of the 8 heads
  (a 128-wide slice of the 512 hidden features), then the partial
  out-projection  attn_out_slice @ Wo[slice, :].  The host sums the 4
  partials per batch (the "all-reduce"); bo is added on the hp==0 core.

Performance structure:
  - Inputs land in SBUF via 17 LARGE DMAs on the sync HWDGE ring
    (per-dma_start issue cost is ~0.6us on the issuing engine, so small
    DMAs serialize); x before context.
  - fp8e4m3 input option halves the input-DMA wall that gates the first
    attention matmul (q needs ALL of x, k needs ALL of context).
  - ~4us of row-group-alternating dummy matmuls at t=0 warm the PE HAM
    clock gate (cold PE = 1.2 GHz, warm = 2.4 GHz).
  - k/v projections keep 8 PSUM accumulators live (acc ring + st-slot
    halves + oaug ring) so the contraction chunks are visited in
    DMA-arrival order.
  - The body is ScalarE-bound (64 exp ACTIVATEs of [128,1024] at
    ~1.1us each): all other PE work (qproj of the NEXT n-chunk,
    outproj of the PREVIOUS one) is interleaved one matmul per
    attention step so the exp stream never pauses.
  - fin() is all-bf16: SBUF->SBUF repartition DMAs + a bf16 PE
    broadcast matmul (an fp32 matmul lowers to two ~1us passes!).
  - Partial outputs are written as f16 (halves output DMA); the host
    sums the 4 partials per batch in f32.
"""

import numpy as np

import concourse.bass as bass
import concourse.tile as tile
from concourse import bacc, mybir
from concourse.masks import make_identity

F32 = mybir.dt.float32
F16 = mybir.dt.float16
BF16 = mybir.dt.bfloat16
F8 = mybir.dt.float8e4

USE_FP8_INPUTS = False   # fp8 x/context fails accuracy: dot-product error does not average down
AT = F8 if USE_FP8_INPUTS else BF16
VPAD = 72                # PV weight row padded to 16B-aligned stride (bf16)

D = 1024      # model dim (contraction for projections)
SEQ = 2048    # n == m
F = 128       # features per core (2 heads x 64)
DH = 64       # head dim
NS = SEQ // 512   # 4 n-chunks of 512
NK = D // 128     # 8 contraction chunks
NM = SEQ // 128   # 16 m-chunks of 128
SCALE = DH ** -0.5
N_WARM = 24       # HAM warm-up matmuls (bridges to the c0 arrival)


def build_nc():
    nc = bacc.Bacc("TRN2", target_bir_lowering=False, debug=False)

    xT_d = nc.dram_tensor("xT", [D, SEQ], AT, kind="ExternalInput")
    cT_d = nc.dram_tensor("cT", [D, SEQ], AT, kind="ExternalInput")
    # host-packed: [128, 3*NK*128]; block (w, k) holds W_w[k*128:(k+1)*128, :]
    # with the chunk's rows on the partition axis.
    wqkv_d = nc.dram_tensor("wqkv", [128, 3 * NK * 128], AT, kind="ExternalInput")
    wo_d = nc.dram_tensor("wo", [F, D], BF16, kind="ExternalInput")
    out_d = nc.dram_tensor("out_p", [SEQ, D], F16, kind="ExternalOutput")

    with tile.TileContext(nc) as tc:
        _emit(tc, nc, xT_d, cT_d, wqkv_d, wo_d, out_d)
    nc.compile()
    return nc


def _emit(tc, nc, xT_d, cT_d, wqkv_d, wo_d, out_d):
    from contextlib import ExitStack

    ctx = ExitStack()
    wpool = ctx.enter_context(tc.tile_pool(name="wpool", bufs=1))
    big = ctx.enter_context(tc.tile_pool(name="big", bufs=1))
    ptp = ctx.enter_context(tc.tile_pool(name="ptp", bufs=6))
    ostage = ctx.enter_context(tc.tile_pool(name="ostage", bufs=2))
    # PSUM budget (8 banks x 2KB):
    #   st ring  : 2 x [128,1024] f32 = 4 banks
    #   oaug ring: 2 x [65,512]  f32 = 2 banks
    #   acc ring : 2 x [128,512] f32 = 2 banks (kacc/vacc/qacc/tp/bc/ops)
    ps_st = ctx.enter_context(tc.tile_pool(name="ps_st", bufs=2, space="PSUM"))
    ps_acc = ctx.enter_context(tc.tile_pool(name="ps_acc", bufs=2, space="PSUM"))
    ps_oaug = ctx.enter_context(tc.tile_pool(name="ps_oaug", bufs=2, space="PSUM"))

    # ---- constants ----
    ident = wpool.tile([128, 128], BF16, name="ident")
    make_identity(nc, ident)
    zbias = wpool.tile([128, 1], F32, name="zbias")
    nc.vector.memset(zbias, 0.0)
    ones64 = wpool.tile([1, DH], BF16, name="ones64")
    nc.vector.memset(ones64, 1.0)
    warm = wpool.tile([128, 512], BF16, name="warm")
    nc.vector.memset(warm, 0.0)

    # ---- input DMAs: one sync-ring queue, program order = arrival order ----
    wqkv_s = wpool.tile([128, 3, NK, 128], AT, name="wqkv_s")
    nc.scalar.dma_start(out=wqkv_s.rearrange("p a b c -> p (a b c)"), in_=wqkv_d.ap())
    cS = wpool.tile([128, NK, SEQ], AT, name="cS")
    for k in range(2):  # first chunks small so the k/v chase starts early
        nc.sync.dma_start(
            out=cS[:, k, :], in_=cT_d.ap()[k * 128 : (k + 1) * 128, :]
        )
    for k in range(2, NK, 2):
        nc.sync.dma_start(
            out=cS[:, k : k + 2, :],
            in_=cT_d.ap()[k * 128 : (k + 2) * 128, :].rearrange(
                "(j p) c -> p j c", j=2
            ),
        )
    xS = wpool.tile([128, NK, SEQ], AT, name="xS")
    for k in range(0, NK, 2):
        nc.sync.dma_start(
            out=xS[:, k : k + 2, :],
            in_=xT_d.ap()[k * 128 : (k + 2) * 128, :].rearrange(
                "(j p) c -> p j c", j=2
            ),
        )
    wo_s = wpool.tile([128, D], BF16, name="wo_s")
    nc.sync.dma_start(out=wo_s, in_=wo_d.ap())

    # ---- persistent activations ----
    qT = big.tile([128, SEQ], BF16, name="qT", tag="qT")
    kT = big.tile([128, SEQ], BF16, name="kT", tag="kT")
    vT = big.tile([128, SEQ], BF16, name="vT", tag="vT")
    OT = big.tile([128, SEQ], BF16, name="OT", tag="OT")
    # V per head+m-chunk, with a ones column (65th) that accumulates the
    # softmax denominators during the PV matmul.
    Vall = big.tile([128, 2, NM, VPAD], BF16, name="Vall", tag="Vall")
    ones_sb = wpool.tile([128, 2 * NM], F32, name="ones_sb")
    nc.vector.memset(ones_sb, 1.0)
    nc.vector.tensor_copy(
        out=Vall[:, :, :, DH : DH + 1],
        in_=ones_sb.rearrange("p (h m o) -> p h m o", h=2, o=1),
    )
    zpad_sb = wpool.tile([128, 2 * NM * (VPAD - DH - 1)], F32, name="zpad_sb")
    nc.vector.memset(zpad_sb, 0.0)
    nc.vector.tensor_copy(
        out=Vall[:, :, :, DH + 1 :],
        in_=zpad_sb.rearrange("p (h m o) -> p h m o", h=2, m=NM),
    )

    # ---- HAM warm-up: >=3.4us of sustained PE streaming with no DMA deps.
    # Row-group alternation lets each LDWEIGHTS overlap the other group's
    # in-flight matmul, so the PE array streams continuously.
    wps = ps_st.tile([128, 1024], F32, name="wps", tag="st")
    for i in range(N_WARM):
        h = i % 2
        nc.tensor.matmul(
            wps[:, 512 * h : 512 * (h + 1)],
            warm[h * 64 : (h + 1) * 64, 0:128],
            warm[h * 64 : (h + 1) * 64, 0:512],
            start=True, stop=True, tile_position=(h * 64, 0),
        )

    # ---- projections ----
    # q for ALL 4 n-chunks chases the xS DMAs with 4 live accumulators
    # (acc ring + oaug ring; kv then reuses those slots).  k/v keeps all
    # 8 accumulators live so the cS chunks are visited in arrival order.
    def qproj_all():
        qa = ps_acc.tile([128, 512], F32, name="qa", tag="acc")
        qb = ps_acc.tile([128, 512], F32, name="qb", tag="acc")
        qc = ps_oaug.tile([128, 512], F32, name="qc", tag="oaug")
        qd = ps_oaug.tile([128, 512], F32, name="qd", tag="oaug")
        accs = [qa, qb, qc, qd]
        for k in range(NK):
            for s in range(4):
                nc.tensor.matmul(
                    accs[s], wqkv_s[:, 0, k, :], xS[:, k, s * 512 : (s + 1) * 512],
                    start=(k == 0), stop=(k == NK - 1),
                )
        for s in range(4):
            eng = nc.vector.tensor_copy if s % 2 == 0 else nc.scalar.copy
            eng(out=qT[:, s * 512 : (s + 1) * 512], in_=accs[s])

    def kvproj_all():
        """k/v for all m in one k-major chase over the cS chunks; the
        evacuations run g=3 first (frees the oaug ring for mk_oaug(0)
        before the attention PV stream reaches the PE queue head)."""
        k0 = ps_acc.tile([128, 512], F32, name="k0", tag="acc")
        v0 = ps_acc.tile([128, 512], F32, name="v0", tag="acc")
        kv1 = ps_st.tile([128, 1024], F32, name="kv1", tag="st")
        kv2 = ps_st.tile([128, 1024], F32, name="kv2", tag="st")
        k3 = ps_oaug.tile([128, 512], F32, name="k3", tag="oaug")
        v3 = ps_oaug.tile([128, 512], F32, name="v3", tag="oaug")
        kaccs = [k0, kv1[:, 0:512], kv2[:, 0:512], k3]
        vaccs = [v0, kv1[:, 512:1024], kv2[:, 512:1024], v3]
        for k in range(NK):
            for g in range(4):
                nc.tensor.matmul(
                    kaccs[g], wqkv_s[:, 1, k, :], cS[:, k, g * 512 : (g + 1) * 512],
                    start=(k == 0), stop=(k == NK - 1),
                )
            for g in range(4):
                nc.tensor.matmul(
                    vaccs[g], wqkv_s[:, 2, k, :], cS[:, k, g * 512 : (g + 1) * 512],
                    start=(k == 0), stop=(k == NK - 1),
                )
        for g in (3, 0, 1, 2):
            nc.vector.tensor_copy(out=kT[:, g * 512 : (g + 1) * 512], in_=kaccs[g])
            nc.scalar.copy(out=vT[:, g * 512 : (g + 1) * 512], in_=vaccs[g])

    def vtrans(g):
        """Vall[:, h, mc, 0:64] = vT[h*64:(h+1)*64, mc*128:(mc+1)*128].T
        Both heads in one [128,128] PE transpose."""
        for mc in range(4 * g, 4 * g + 4):
            tp = ps_acc.tile([128, 128], BF16, name="tp", tag="acc")
            nc.tensor.transpose(tp, vT[:, mc * 128 : (mc + 1) * 128], ident)
            nc.vector.tensor_copy(
                out=Vall[:, :, mc, 0:DH],
                in_=tp.rearrange("p (h d) -> p h d", h=2),
            )

    def attn_one(s, oaug, mc):
        """One m-chunk of attention for n-chunk s."""
        n0, n1 = s * 512, (s + 1) * 512
        m0, m1 = mc * 128, (mc + 1) * 128
        st = ps_st.tile([128, 1024], F32, name="st", tag="st")
        nc.tensor.matmul(
            st[:, 0:512], kT[0:DH, m0:m1], qT[0:DH, n0:n1],
            start=True, stop=True, tile_position=(0, 0),
        )
        nc.tensor.matmul(
            st[:, 512:1024], kT[DH:128, m0:m1], qT[DH:128, n0:n1],
            start=True, stop=True, tile_position=(64, 0),
        )
        pt = ptp.tile([128, 1024], BF16, name="pt", tag="pt")
        nc.scalar.activation(
            out=pt, in_=st,
            func=mybir.ActivationFunctionType.Exp,
            bias=zbias, scale=SCALE,
        )
        nc.tensor.matmul(
            oaug[0], Vall[:, 0, mc, 0 : DH + 1], pt[:, 0:512],
            start=(mc == 0), stop=(mc == NM - 1),
        )
        nc.tensor.matmul(
            oaug[1], Vall[:, 1, mc, 0 : DH + 1], pt[:, 512:1024],
            start=(mc == 0), stop=(mc == NM - 1),
        )

    fin_state = {}

    def attn_pipe2(s, oaug):
        """First two m-chunks of a phase with both St/exp pairs issued
        ahead of the PVs, so the PE queue head doesn't block on
        fin_pre's oaug evacuations (PV mc0 start=True waits on them)."""
        n0, n1 = s * 512, (s + 1) * 512
        pts = []
        for mc in (0, 1):
            m0, m1 = mc * 128, (mc + 1) * 128
            st = ps_st.tile([128, 1024], F32, name="st", tag="st")
            nc.tensor.matmul(
                st[:, 0:512], kT[0:DH, m0:m1], qT[0:DH, n0:n1],
                start=True, stop=True, tile_position=(0, 0),
            )
            nc.tensor.matmul(
                st[:, 512:1024], kT[DH:128, m0:m1], qT[DH:128, n0:n1],
                start=True, stop=True, tile_position=(64, 0),
            )
            pt = ptp.tile([128, 1024], BF16, name="pt", tag="pt")
            nc.scalar.activation(
                out=pt, in_=st,
                func=mybir.ActivationFunctionType.Exp,
                bias=zbias, scale=SCALE,
            )
            pts.append(pt)
        for mc in (0, 1):
            nc.tensor.matmul(
                oaug[0], Vall[:, 0, mc, 0 : DH + 1], pts[mc][:, 0:512],
                start=(mc == 0), stop=False,
            )
            nc.tensor.matmul(
                oaug[1], Vall[:, 1, mc, 0 : DH + 1], pts[mc][:, 512:1024],
                start=(mc == 0), stop=False,
            )

    def fin_pre(s, oaug, last=False):
        """Start softmax-denominator normalization for n-chunk s: evacuate
        oaug, repartition the [1,512] denominator row to [128,4]
        (SBUF->SBUF DMA), reciprocal on all DVE lanes, DMA back to a
        bf16 [1,512] row.  All DMAs for both heads are interleaved so
        the two chains pipeline.  No PE work here -- the bc broadcast
        matmuls are emitted later (fin_bc) so the PE queue head never
        blocks on this chain's DMA latency."""
        sbs, recs = [], []
        for h in range(2):
            oaug_sb = ostage.tile([DH + 1, 512], F32, name="oaug_sb",
                                  tag="oaug_sb", bufs=2)
            if last and h == 0:
                nc.scalar.copy(out=oaug_sb, in_=oaug[h])
            else:
                nc.vector.tensor_copy(out=oaug_sb, in_=oaug[h])
            sbs.append(oaug_sb)
        dens = []
        for h in range(2):
            den_p = ostage.tile([128, 4], F32, name="den_p", tag="den_p", bufs=2)
            nc.sync.dma_start(out=den_p, in_=sbs[h][DH : DH + 1, :])
            dens.append(den_p)
        rps = []
        for h in range(2):
            rec_p = ostage.tile([128, 4], BF16, name="rec_p", tag="rec_p", bufs=2)
            with nc.allow_low_precision(reason="bf16 softmax denominators"):
                nc.vector.reciprocal(out=rec_p, in_=dens[h])
            rps.append(rec_p)
        for h in range(2):
            rec_row = ostage.tile([1, 512], BF16, name="rec_row", tag="rec_row",
                                  bufs=2)
            nc.sync.dma_start(out=rec_row, in_=rps[h])
            recs.append(rec_row)
        fin_state[s] = (sbs, recs)

    def fin_bc(s):
        """Finish fin: broadcast the reciprocal row to 64 partitions with
        a bf16 PE matmul and scale oaug into OT."""
        n0, n1 = s * 512, (s + 1) * 512
        sbs, recs = fin_state.pop(s)
        for h in range(2):
            bc = ps_acc.tile([DH, 512], F32, name="bc", tag="acc")
            nc.tensor.matmul(bc, ones64, recs[h], start=True, stop=True)
            nc.vector.tensor_mul(
                out=OT[h * DH : (h + 1) * DH, n0:n1],
                in0=sbs[h][0:DH, :],
                in1=bc,
            )

    def outproj_piece(s, i, tail=False):
        """Piece i (of 8) of the out-projection for n-chunk s.  The bias
        bo is added on the host during the partial-sum gather."""
        nt = s * 4 + i // 2
        half = i % 2
        c0, c1 = half * 512, (half + 1) * 512
        ops = ps_acc.tile([128, 512], F32, name="ops", tag="acc")
        nc.tensor.matmul(
            ops, OT[:, nt * 128 : (nt + 1) * 128], wo_s[:, c0:c1],
            start=True, stop=True,
        )
        osb = ostage.tile([128, 512], F16, name="osb", tag="osb", bufs=4)
        with nc.allow_low_precision(reason="f16 partial outputs"):
            if tail and i % 2 == 1:
                nc.scalar.copy(out=osb, in_=ops)
            else:
                nc.vector.tensor_copy(out=osb, in_=ops)
        if tail:
            eng = nc.sync if i % 2 == 0 else nc.gpsimd
        else:
            eng = nc.sync if i % 2 == 1 else nc.gpsimd
        eng.dma_start(out=out_d.ap()[nt * 128 : (nt + 1) * 128, c0:c1], in_=osb)

    # ---- schedule ----
    def mk_oaug(s):
        return [
            ps_oaug.tile([DH + 1, 512], F32, name=f"oaug{s}_{h}", tag="oaug")
            for h in range(2)
        ]

    wf_tile = []

    def warm_fill(n):
        """Dummy matmuls into a free st-ring slot to keep/restore HAM."""
        if not wf_tile:
            wf_tile.append(ps_st.tile([128, 1024], F32, name="wf", tag="st"))
        wf = wf_tile[0]
        for i in range(n):
            h = i % 2
            nc.tensor.matmul(
                wf[:, 512 * h : 512 * (h + 1)],
                warm[h * 64 : (h + 1) * 64, 0:128],
                warm[h * 64 : (h + 1) * 64, 0:512],
                start=True, stop=True, tile_position=(h * 64, 0),
            )

    kvproj_all()
    qproj_all()
    vtrans(0)
    oaug_cur = mk_oaug(0)
    for g in range(4):
        if g < 3:
            vtrans(g + 1)
        for mc in range(4 * g, 4 * g + 4):
            attn_one(0, oaug_cur, mc)
    for s in range(1, NS):
        oaug_next = mk_oaug(s)
        fin_pre(s - 1, oaug_cur)
        attn_pipe2(s, oaug_next)
        for mc in range(2, 4):
            attn_one(s, oaug_next, mc)
        fin_bc(s - 1)
        piece_at = {5: 0, 6: 1, 7: 2, 8: 3, 9: 4, 10: 5, 11: 6, 12: 7}
        for mc in range(4, NM):
            attn_one(s, oaug_next, mc)
            if mc in piece_at:
                outproj_piece(s - 1, piece_at[mc])
        oaug_cur = oaug_next
    fin_pre(NS - 1, oaug_cur, last=True)
    warm_fill(48)
    fin_bc(NS - 1)
    for i in range(8):
        outproj_piece(NS - 1, i, tail=True)
        if i < 7:
            warm_fill(2)

    ctx.close()


_NC = None


def _get_nc():
    global _NC
    if _NC is None:
        _NC = build_nc()
    return _NC


def _np_at():
    import ml_dtypes

    return ml_dtypes.float8_e4m3 if USE_FP8_INPUTS else ml_dtypes.bfloat16


def _swizzle(w):
    """[1024, 128] -> [128, 8*128]: chunk k of the contraction dim lands in
    column block k with the chunk's rows on the partition axis."""
    return (
        np.asarray(w, np.float32).reshape(NK, 128, F).transpose(1, 0, 2)
        .reshape(128, NK * F)
    )


def shard_inputs(x, context, Wq, Wk, Wv, Wo, bo):
    import ml_dtypes

    x = np.asarray(x, np.float32)
    context = np.asarray(context, np.float32)
    Wq = np.asarray(Wq, np.float32)
    Wk = np.asarray(Wk, np.float32)
    Wv = np.asarray(Wv, np.float32)
    Wo = np.asarray(Wo, np.float32)
    bo = np.asarray(bo, np.float32)

    at = _np_at()
    xT = [np.ascontiguousarray(x[b].T).astype(at) for b in range(x.shape[0])]
    cT = [np.ascontiguousarray(context[b].T).astype(at) for b in range(context.shape[0])]
    in_maps = []
    for c in range(8):
        b, hp = divmod(c, 4)
        f0 = hp * F
        wqkv = np.ascontiguousarray(
            np.concatenate(
                [
                    _swizzle(Wq[:, f0 : f0 + F]),
                    _swizzle(Wk[:, f0 : f0 + F]),
                    _swizzle(Wv[:, f0 : f0 + F]),
                ],
                axis=1,
            )
        ).astype(at)
        in_maps.append(
            {
                "xT": xT[b],
                "cT": cT[b],
                "wqkv": wqkv,
                "wo": np.ascontiguousarray(Wo[f0 : f0 + F, :]).astype(
                    ml_dtypes.bfloat16
                ),
            }
        )
    return in_maps


def kernel(x, context, Wq, Wk, Wv, Wo, bo):
    from concourse.bass_utils import run_bass_kernel_spmd

    in_maps = shard_inputs(x, context, Wq, Wk, Wv, Wo, bo)
    nc = _get_nc()
    res = run_bass_kernel_spmd(nc, in_maps, list(range(8)))
    out = np.zeros((2, SEQ, D), np.float32)
    for c in range(8):
        out[c // 4] += np.asarray(res.results[c]["out_p"], np.float32)
    out += np.asarray(bo, np.float32)  # bias folded into the gather
    return out


# revision 24
# speedup vs baseline: 1.0357x; 1.0357x over previous
"""Cross-attention kernel for Trainium2, 8 NeuronCores.

Sharding (data + head parallel, per the problem's sharding hint):
  core c in 0..7 -> batch b = c // 4, head-pair hp = c % 4.
  Each core computes attention for its batch with 2 of the 8 heads
  (a 128-wide slice of the 512 hidden features), then the partial
  out-projection  attn_out_slice @ Wo[slice, :].  The host sums the 4
  partials per batch (the "all-reduce"); bo is added on the hp==0 core.

Performance structure:
  - Inputs land in SBUF via 17 LARGE DMAs on the sync HWDGE ring
    (per-dma_start issue cost is ~0.6us on the issuing engine, so small
    DMAs serialize); x before context.
  - fp8e4m3 input option halves the input-DMA wall that gates the first
    attention matmul (q needs ALL of x, k needs ALL of context).
  - ~4us of row-group-alternating dummy matmuls at t=0 warm the PE HAM
    clock gate (cold PE = 1.2 GHz, warm = 2.4 GHz).
  - k/v projections keep 8 PSUM accumulators live (acc ring + st-slot
    halves + oaug ring) so the contraction chunks are visited in
    DMA-arrival order.
  - The body is ScalarE-bound (64 exp ACTIVATEs of [128,1024] at
    ~1.1us each): all other PE work (qproj of the NEXT n-chunk,
    outproj of the PREVIOUS one) is interleaved one matmul per
    attention step so the exp stream never pauses.
  - fin() is all-bf16: SBUF->SBUF repartition DMAs + a bf16 PE
    broadcast matmul (an fp32 matmul lowers to two ~1us passes!).
  - Partial outputs are written as f16 (halves output DMA); the host
    sums the 4 partials per batch in f32.
"""

import numpy as np

import concourse.bass as bass
import concourse.tile as tile
from concourse import bacc, mybir
from concourse.masks import make_identity

F32 = mybir.dt.float32
F16 = mybir.dt.float16
BF16 = mybir.dt.bfloat16
F8 = mybir.dt.float8e4

USE_FP8_INPUTS = False   # fp8 x/context fails accuracy: dot-product error does not average down
AT = F8 if USE_FP8_INPUTS else BF16
VPAD = 72                # PV weight row padded to 16B-aligned stride (bf16)

D = 1024      # model dim (contraction for projections)
SEQ = 2048    # n == m
F = 128       # features per core (2 heads x 64)
DH = 64       # head dim
NS = SEQ // 512   # 4 n-chunks of 512
NK = D // 128     # 8 contraction chunks
NM = SEQ // 128   # 16 m-chunks of 128
SCALE = DH ** -0.5
N_WARM = 24       # HAM warm-up matmuls (bridges to the c0 arrival)


def build_nc():
    nc = bacc.Bacc("TRN2", target_bir_lowering=False, debug=False)

    xT_d = nc.dram_tensor("xT", [D, SEQ], AT, kind="ExternalInput")
    cT_d = nc.dram_tensor("cT", [D, SEQ], AT, kind="ExternalInput")
    # host-packed: [128, 3*NK*128]; block (w, k) holds W_w[k*128:(k+1)*128, :]
    # with the chunk's rows on the partition axis.
    wqkv_d = nc.dram_tensor("wqkv", [128, 3 * NK * 128], AT, kind="ExternalInput")
    wo_d = nc.dram_tensor("wo", [F, D], BF16, kind="ExternalInput")
    out_d = nc.dram_tensor("out_p", [SEQ, D], F16, kind="ExternalOutput")

    with tile.TileContext(nc) as tc:
        _emit(tc, nc, xT_d, cT_d, wqkv_d, wo_d, out_d)
    nc.compile()
    return nc


def _emit(tc, nc, xT_d, cT_d, wqkv_d, wo_d, out_d):
    from contextlib import ExitStack

    ctx = ExitStack()
    wpool = ctx.enter_context(tc.tile_pool(name="wpool", bufs=1))
    big = ctx.enter_context(tc.tile_pool(name="big", bufs=1))
    ptp = ctx.enter_context(tc.tile_pool(name="ptp", bufs=6))
    ostage = ctx.enter_context(tc.tile_pool(name="ostage", bufs=2))
    # PSUM budget (8 banks x 2KB):
    #   st ring  : 2 x [128,1024] f32 = 4 banks
    #   oaug ring: 2 x [65,512]  f32 = 2 banks
    #   acc ring : 2 x [128,512] f32 = 2 banks (kacc/vacc/qacc/tp/bc/ops)
    ps_st = ctx.enter_context(tc.tile_pool(name="ps_st", bufs=2, space="PSUM"))
    ps_acc = ctx.enter_context(tc.tile_pool(name="ps_acc", bufs=2, space="PSUM"))
    ps_oaug = ctx.enter_context(tc.tile_pool(name="ps_oaug", bufs=2, space="PSUM"))

    # ---- constants ----
    ident = wpool.tile([128, 128], BF16, name="ident")
    make_identity(nc, ident)
    zbias = wpool.tile([128, 1], F32, name="zbias")
    nc.vector.memset(zbias, 0.0)
    ones64 = wpool.tile([1, DH], BF16, name="ones64")
    nc.vector.memset(ones64, 1.0)
    warm = wpool.tile([128, 512], BF16, name="warm")
    nc.vector.memset(warm, 0.0)

    # ---- input DMAs: one sync-ring queue, program order = arrival order ----
    wqkv_s = wpool.tile([128, 3, NK, 128], AT, name="wqkv_s")
    nc.scalar.dma_start(out=wqkv_s.rearrange("p a b c -> p (a b c)"), in_=wqkv_d.ap())
    cS = wpool.tile([128, NK, SEQ], AT, name="cS")
    for k in range(2):  # first chunks small so the k/v chase starts early
        nc.sync.dma_start(
            out=cS[:, k, :], in_=cT_d.ap()[k * 128 : (k + 1) * 128, :]
        )
    for k in range(2, NK, 2):
        nc.sync.dma_start(
            out=cS[:, k : k + 2, :],
            in_=cT_d.ap()[k * 128 : (k + 2) * 128, :].rearrange(
                "(j p) c -> p j c", j=2
            ),
        )
    xS = wpool.tile([128, NK, SEQ], AT, name="xS")
    for k in range(0, NK, 2):
        nc.sync.dma_start(
            out=xS[:, k : k + 2, :],
            in_=xT_d.ap()[k * 128 : (k + 2) * 128, :].rearrange(
                "(j p) c -> p j c", j=2
            ),
        )
    wo_s = wpool.tile([128, D], BF16, name="wo_s")
    nc.sync.dma_start(out=wo_s, in_=wo_d.ap())

    # ---- persistent activations ----
    qT = big.tile([128, SEQ], BF16, name="qT", tag="qT")
    kT = big.tile([128, SEQ], BF16, name="kT", tag="kT")
    vT = big.tile([128, SEQ], BF16, name="vT", tag="vT")
    OT = big.tile([128, SEQ], BF16, name="OT", tag="OT")
    # V per head+m-chunk, with a ones column (65th) that accumulates the
    # softmax denominators during the PV matmul.
    Vall = big.tile([128, 2, NM, VPAD], BF16, name="Vall", tag="Vall")
    ones_sb = wpool.tile([128, 2 * NM], F32, name="ones_sb")
    nc.vector.memset(ones_sb, 1.0)
    nc.vector.tensor_copy(
        out=Vall[:, :, :, DH : DH + 1],
        in_=ones_sb.rearrange("p (h m o) -> p h m o", h=2, o=1),
    )
    zpad_sb = wpool.tile([128, 2 * NM * (VPAD - DH - 1)], F32, name="zpad_sb")
    nc.vector.memset(zpad_sb, 0.0)
    nc.vector.tensor_copy(
        out=Vall[:, :, :, DH + 1 :],
        in_=zpad_sb.rearrange("p (h m o) -> p h m o", h=2, m=NM),
    )

    # ---- HAM warm-up: >=3.4us of sustained PE streaming with no DMA deps.
    # Row-group alternation lets each LDWEIGHTS overlap the other group's
    # in-flight matmul, so the PE array streams continuously.
    wps = ps_st.tile([128, 1024], F32, name="wps", tag="st")
    for i in range(N_WARM):
        h = i % 2
        nc.tensor.matmul(
            wps[:, 512 * h : 512 * (h + 1)],
            warm[h * 64 : (h + 1) * 64, 0:128],
            warm[h * 64 : (h + 1) * 64, 0:512],
            start=True, stop=True, tile_position=(h * 64, 0),
        )

    # ---- projections ----
    # q for ALL 4 n-chunks chases the xS DMAs with 4 live accumulators
    # (acc ring + oaug ring; kv then reuses those slots).  k/v keeps all
    # 8 accumulators live so the cS chunks are visited in arrival order.
    def qproj_all():
        qa = ps_acc.tile([128, 512], F32, name="qa", tag="acc")
        qb = ps_acc.tile([128, 512], F32, name="qb", tag="acc")
        qc = ps_oaug.tile([128, 512], F32, name="qc", tag="oaug")
        qd = ps_oaug.tile([128, 512], F32, name="qd", tag="oaug")
        accs = [qa, qb, qc, qd]
        for k in range(NK):
            for s in range(4):
                nc.tensor.matmul(
                    accs[s], wqkv_s[:, 0, k, :], xS[:, k, s * 512 : (s + 1) * 512],
                    start=(k == 0), stop=(k == NK - 1),
                )
        for s in range(4):
            eng = nc.vector.tensor_copy if s % 2 == 0 else nc.scalar.copy
            eng(out=qT[:, s * 512 : (s + 1) * 512], in_=accs[s])

    def kvproj_all():
        """k/v for all m in one k-major chase over the cS chunks; the
        evacuations run g=3 first (frees the oaug ring for mk_oaug(0)
        before the attention PV stream reaches the PE queue head)."""
        k0 = ps_acc.tile([128, 512], F32, name="k0", tag="acc")
        v0 = ps_acc.tile([128, 512], F32, name="v0", tag="acc")
        kv1 = ps_st.tile([128, 1024], F32, name="kv1", tag="st")
        kv2 = ps_st.tile([128, 1024], F32, name="kv2", tag="st")
        k3 = ps_oaug.tile([128, 512], F32, name="k3", tag="oaug")
        v3 = ps_oaug.tile([128, 512], F32, name="v3", tag="oaug")
        kaccs = [k0, kv1[:, 0:512], kv2[:, 0:512], k3]
        vaccs = [v0, kv1[:, 512:1024], kv2[:, 512:1024], v3]
        for k in range(NK):
            for g in range(4):
                nc.tensor.matmul(
                    kaccs[g], wqkv_s[:, 1, k, :], cS[:, k, g * 512 : (g + 1) * 512],
                    start=(k == 0), stop=(k == NK - 1),
                )
            for g in range(4):
                nc.tensor.matmul(
                    vaccs[g], wqkv_s[:, 2, k, :], cS[:, k, g * 512 : (g + 1) * 512],
                    start=(k == 0), stop=(k == NK - 1),
                )
        for g in (3, 0, 1, 2):
            nc.vector.tensor_copy(out=kT[:, g * 512 : (g + 1) * 512], in_=kaccs[g])
            nc.scalar.copy(out=vT[:, g * 512 : (g + 1) * 512], in_=vaccs[g])

    def vtrans(g):
        """Vall[:, h, mc, 0:64] = vT[h*64:(h+1)*64, mc*128:(mc+1)*128].T
        Both heads in one [128,128] PE transpose."""
        for mc in range(4 * g, 4 * g + 4):
            tp = ps_acc.tile([128, 128], BF16, name="tp", tag="acc")
            nc.tensor.transpose(tp, vT[:, mc * 128 : (mc + 1) * 128], ident)
            nc.vector.tensor_copy(
                out=Vall[:, :, mc, 0:DH],
                in_=tp.rearrange("p (h d) -> p h d", h=2),
            )

    def attn_one(s, oaug, mc):
        """One m-chunk of attention for n-chunk s."""
        n0, n1 = s * 512, (s + 1) * 512
        m0, m1 = mc * 128, (mc + 1) * 128
        st = ps_st.tile([128, 1024], F32, name="st", tag="st")
        nc.tensor.matmul(
            st[:, 0:512], kT[0:DH, m0:m1], qT[0:DH, n0:n1],
            start=True, stop=True, tile_position=(0, 0),
        )
        nc.tensor.matmul(
            st[:, 512:1024], kT[DH:128, m0:m1], qT[DH:128, n0:n1],
            start=True, stop=True, tile_position=(64, 0),
        )
        pt = ptp.tile([128, 1024], BF16, name="pt", tag="pt")
        nc.scalar.activation(
            out=pt, in_=st,
            func=mybir.ActivationFunctionType.Exp,
            bias=zbias, scale=SCALE,
        )
        nc.tensor.matmul(
            oaug[0], Vall[:, 0, mc, 0 : DH + 1], pt[:, 0:512],
            start=(mc == 0), stop=(mc == NM - 1),
        )
        nc.tensor.matmul(
            oaug[1], Vall[:, 1, mc, 0 : DH + 1], pt[:, 512:1024],
            start=(mc == 0), stop=(mc == NM - 1),
        )

    fin_state = {}

    pt_live = {}

    def emit_st(s, mc):
        """Score matmuls + exp for one (s, mc) slot."""
        n0, n1 = s * 512, (s + 1) * 512
        m0, m1 = mc * 128, (mc + 1) * 128
        st = ps_st.tile([128, 1024], F32, name="st", tag="st")
        nc.tensor.matmul(
            st[:, 0:512], kT[0:DH, m0:m1], qT[0:DH, n0:n1],
            start=True, stop=True, tile_position=(0, 0),
        )
        nc.tensor.matmul(
            st[:, 512:1024], kT[DH:128, m0:m1], qT[DH:128, n0:n1],
            start=True, stop=True, tile_position=(64, 0),
        )
        pt = ptp.tile([128, 1024], BF16, name="pt", tag="pt")
        nc.scalar.activation(
            out=pt, in_=st,
            func=mybir.ActivationFunctionType.Exp,
            bias=zbias, scale=SCALE,
        )
        pt_live[(s, mc)] = pt

    def emit_pv(s, oaug, mc):
        pt = pt_live.pop((s, mc))
        nc.tensor.matmul(
            oaug[0], Vall[:, 0, mc, 0 : DH + 1], pt[:, 0:512],
            start=(mc == 0), stop=(mc == NM - 1),
        )
        nc.tensor.matmul(
            oaug[1], Vall[:, 1, mc, 0 : DH + 1], pt[:, 512:1024],
            start=(mc == 0), stop=(mc == NM - 1),
        )

    def attn_pipe2(s, oaug):
        """First two m-chunks of a phase with both St/exp pairs issued
        ahead of the PVs, so the PE queue head doesn't block on
        fin_pre's oaug evacuations (PV mc0 start=True waits on them)."""
        n0, n1 = s * 512, (s + 1) * 512
        pts = []
        for mc in (0, 1):
            m0, m1 = mc * 128, (mc + 1) * 128
            st = ps_st.tile([128, 1024], F32, name="st", tag="st")
            nc.tensor.matmul(
                st[:, 0:512], kT[0:DH, m0:m1], qT[0:DH, n0:n1],
                start=True, stop=True, tile_position=(0, 0),
            )
            nc.tensor.matmul(
                st[:, 512:1024], kT[DH:128, m0:m1], qT[DH:128, n0:n1],
                start=True, stop=True, tile_position=(64, 0),
            )
            pt = ptp.tile([128, 1024], BF16, name="pt", tag="pt")
            nc.scalar.activation(
                out=pt, in_=st,
                func=mybir.ActivationFunctionType.Exp,
                bias=zbias, scale=SCALE,
            )
            pts.append(pt)
        for mc in (0, 1):
            nc.tensor.matmul(
                oaug[0], Vall[:, 0, mc, 0 : DH + 1], pts[mc][:, 0:512],
                start=(mc == 0), stop=False,
            )
            nc.tensor.matmul(
                oaug[1], Vall[:, 1, mc, 0 : DH + 1], pts[mc][:, 512:1024],
                start=(mc == 0), stop=False,
            )

    def fin_pre(s, oaug, last=False):
        """Start softmax-denominator normalization for n-chunk s: evacuate
        oaug, repartition the [1,512] denominator row to [128,4]
        (SBUF->SBUF DMA), reciprocal on all DVE lanes, DMA back to a
        bf16 [1,512] row.  All DMAs for both heads are interleaved so
        the two chains pipeline.  No PE work here -- the bc broadcast
        matmuls are emitted later (fin_bc) so the PE queue head never
        blocks on this chain's DMA latency."""
        sbs, recs = [], []
        for h in range(2):
            oaug_sb = ostage.tile([DH + 1, 512], F32, name="oaug_sb",
                                  tag="oaug_sb", bufs=2)
            if last and h == 0:
                nc.scalar.copy(out=oaug_sb, in_=oaug[h])
            else:
                nc.vector.tensor_copy(out=oaug_sb, in_=oaug[h])
            sbs.append(oaug_sb)
        dens = []
        for h in range(2):
            den_p = ostage.tile([128, 4], F32, name="den_p", tag="den_p", bufs=2)
            nc.sync.dma_start(out=den_p, in_=sbs[h][DH : DH + 1, :])
            dens.append(den_p)
        rps = []
        for h in range(2):
            rec_p = ostage.tile([128, 4], BF16, name="rec_p", tag="rec_p", bufs=2)
            with nc.allow_low_precision(reason="bf16 softmax denominators"):
                nc.vector.reciprocal(out=rec_p, in_=dens[h])
            rps.append(rec_p)
        for h in range(2):
            rec_row = ostage.tile([1, 512], BF16, name="rec_row", tag="rec_row",
                                  bufs=2)
            nc.sync.dma_start(out=rec_row, in_=rps[h])
            recs.append(rec_row)
        fin_state[s] = (sbs, recs)

    def fin_bc(s):
        """Finish fin: broadcast the reciprocal row to 64 partitions with
        a bf16 PE matmul and scale oaug into OT."""
        n0, n1 = s * 512, (s + 1) * 512
        sbs, recs = fin_state.pop(s)
        for h in range(2):
            bc = ps_acc.tile([DH, 512], F32, name="bc", tag="acc")
            nc.tensor.matmul(bc, ones64, recs[h], start=True, stop=True)
            nc.vector.tensor_mul(
                out=OT[h * DH : (h + 1) * DH, n0:n1],
                in0=sbs[h][0:DH, :],
                in1=bc,
            )

    def outproj_piece(s, i, tail=False):
        """Piece i (of 8) of the out-projection for n-chunk s.  The bias
        bo is added on the host during the partial-sum gather."""
        nt = s * 4 + i // 2
        half = i % 2
        c0, c1 = half * 512, (half + 1) * 512
        ops = ps_acc.tile([128, 512], F32, name="ops", tag="acc")
        nc.tensor.matmul(
            ops, OT[:, nt * 128 : (nt + 1) * 128], wo_s[:, c0:c1],
            start=True, stop=True,
        )
        osb = ostage.tile([128, 512], F16, name="osb", tag="osb", bufs=4)
        with nc.allow_low_precision(reason="f16 partial outputs"):
            if tail and i % 2 == 1:
                nc.scalar.copy(out=osb, in_=ops)
            else:
                nc.vector.tensor_copy(out=osb, in_=ops)
        if tail:
            eng = nc.sync if i % 2 == 0 else nc.gpsimd
        else:
            eng = nc.sync if i % 2 == 1 else nc.gpsimd
        eng.dma_start(out=out_d.ap()[nt * 128 : (nt + 1) * 128, c0:c1], in_=osb)

    # ---- schedule ----
    def mk_oaug(s):
        return [
            ps_oaug.tile([DH + 1, 512], F32, name=f"oaug{s}_{h}", tag="oaug")
            for h in range(2)
        ]

    wf_tile = []

    def warm_fill(n):
        """Dummy matmuls into a free st-ring slot to keep/restore HAM."""
        if not wf_tile:
            wf_tile.append(ps_st.tile([128, 1024], F32, name="wf", tag="st"))
        wf = wf_tile[0]
        for i in range(n):
            h = i % 2
            nc.tensor.matmul(
                wf[:, 512 * h : 512 * (h + 1)],
                warm[h * 64 : (h + 1) * 64, 0:128],
                warm[h * 64 : (h + 1) * 64, 0:512],
                start=True, stop=True, tile_position=(h * 64, 0),
            )

    kvproj_all()
    qproj_all()
    vtrans(0)
    oaugs = {0: mk_oaug(0)}
    slots = [(s, mc) for s in range(NS) for mc in range(NM)]
    # St/exp for slot i+2 issues right after PV of slot i, so the score
    # matmuls for the next phase run during the current phase's last
    # exps and the exp stream never pauses at phase boundaries.
    emit_st(*slots[0])
    emit_st(*slots[1])
    for i, (s, mc) in enumerate(slots):
        if s == 0 and mc % 4 == 0 and mc < 12:
            vtrans(mc // 4 + 1)
        if s >= 1 and mc == 0:
            fin_pre(s - 1, oaugs[s - 1])
            oaugs[s] = mk_oaug(s)
        if s >= 1 and mc == 4:
            fin_bc(s - 1)
        if s >= 1 and 5 <= mc <= 12:
            outproj_piece(s - 1, mc - 5)
        emit_pv(s, oaugs[s], mc)
        if i + 2 < len(slots):
            emit_st(*slots[i + 2])
    oaug_cur = oaugs[NS - 1]
    fin_pre(NS - 1, oaug_cur, last=True)
    warm_fill(48)
    fin_bc(NS - 1)
    for i in range(8):
        outproj_piece(NS - 1, i, tail=True)
        if i < 7:
            warm_fill(2)

    ctx.close()


_NC = None


def _get_nc():
    global _NC
    if _NC is None:
        _NC = build_nc()
    return _NC


def _np_at():
    import ml_dtypes

    return ml_dtypes.float8_e4m3 if USE_FP8_INPUTS else ml_dtypes.bfloat16


def _swizzle(w):
    """[1024, 128] -> [128, 8*128]: chunk k of the contraction dim lands in
    column block k with the chunk's rows on the partition axis."""
    return (
        np.asarray(w, np.float32).reshape(NK, 128, F).transpose(1, 0, 2)
        .reshape(128, NK * F)
    )


def shard_inputs(x, context, Wq, Wk, Wv, Wo, bo):
    import ml_dtypes

    x = np.asarray(x, np.float32)
    context = np.asarray(context, np.float32)
    Wq = np.asarray(Wq, np.float32)
    Wk = np.asarray(Wk, np.float32)
    Wv = np.asarray(Wv, np.float32)
    Wo = np.asarray(Wo, np.float32)
    bo = np.asarray(bo, np.float32)

    at = _np_at()
    xT = [np.ascontiguousarray(x[b].T).astype(at) for b in range(x.shape[0])]
    cT = [np.ascontiguousarray(context[b].T).astype(at) for b in range(context.shape[0])]
    in_maps = []
    for c in range(8):
        b, hp = divmod(c, 4)
        f0 = hp * F
        wqkv = np.ascontiguousarray(
            np.concatenate(
                [
                    _swizzle(Wq[:, f0 : f0 + F]),
                    _swizzle(Wk[:, f0 : f0 + F]),
                    _swizzle(Wv[:, f0 : f0 + F]),
                ],
                axis=1,
            )
        ).astype(at)
        in_maps.append(
            {
                "xT": xT[b],
                "cT": cT[b],
                "wqkv": wqkv,
                "wo": np.ascontiguousarray(Wo[f0 : f0 + F, :]).astype(
                    ml_dtypes.bfloat16
                ),
            }
        )
    return in_maps


def kernel(x, context, Wq, Wk, Wv, Wo, bo):
    from concourse.bass_utils import run_bass_kernel_spmd

    in_maps = shard_inputs(x, context, Wq, Wk, Wv, Wo, bo)
    nc = _get_nc()
    res = run_bass_kernel_spmd(nc, in_maps, list(range(8)))
    out = np.zeros((2, SEQ, D), np.float32)
    for c in range(8):
        out[c // 4] += np.asarray(res.results[c]["out_p"], np.float32)
    out += np.asarray(bo, np.float32)  # bias folded into the gather
    return out


# revision 25
# speedup vs baseline: 1.0529x; 1.0166x over previous
"""Cross-attention kernel for Trainium2, 8 NeuronCores.

Sharding (data + head parallel, per the problem's sharding hint):
  core c in 0..7 -> batch b = c // 4, head-pair hp = c % 4.
  Each core computes attention for its batch with 2 of the 8 heads
  (a 128-wide slice of the 512 hidden features), then the partial
  out-projection  attn_out_slice @ Wo[slice, :].  The host sums the 4
  partials per batch (the "all-reduce"); bo is added on the hp==0 core.

Performance structure:
  - Inputs land in SBUF via 17 LARGE DMAs on the sync HWDGE ring
    (per-dma_start issue cost is ~0.6us on the issuing engine, so small
    DMAs serialize); x before context.
  - fp8e4m3 input option halves the input-DMA wall that gates the first
    attention matmul (q needs ALL of x, k needs ALL of context).
  - ~4us of row-group-alternating dummy matmuls at t=0 warm the PE HAM
    clock gate (cold PE = 1.2 GHz, warm = 2.4 GHz).
  - k/v projections keep 8 PSUM accumulators live (acc ring + st-slot
    halves + oaug ring) so the contraction chunks are visited in
    DMA-arrival order.
  - The body is ScalarE-bound (64 exp ACTIVATEs of [128,1024] at
    ~1.1us each): all other PE work (qproj of the NEXT n-chunk,
    outproj of the PREVIOUS one) is interleaved one matmul per
    attention step so the exp stream never pauses.
  - fin() is all-bf16: SBUF->SBUF repartition DMAs + a bf16 PE
    broadcast matmul (an fp32 matmul lowers to two ~1us passes!).
  - Partial outputs are written as f16 (halves output DMA); the host
    sums the 4 partials per batch in f32.
"""

import numpy as np

import concourse.bass as bass
import concourse.tile as tile
from concourse import bacc, mybir
from concourse.masks import make_identity

F32 = mybir.dt.float32
F16 = mybir.dt.float16
BF16 = mybir.dt.bfloat16
F8 = mybir.dt.float8e4

USE_FP8_INPUTS = False   # fp8 x/context fails accuracy: dot-product error does not average down
AT = F8 if USE_FP8_INPUTS else BF16
VPAD = 72                # PV weight row padded to 16B-aligned stride (bf16)

D = 1024      # model dim (contraction for projections)
SEQ = 2048    # n == m
F = 128       # features per core (2 heads x 64)
DH = 64       # head dim
NS = SEQ // 512   # 4 n-chunks of 512
NK = D // 128     # 8 contraction chunks
NM = SEQ // 128   # 16 m-chunks of 128
SCALE = DH ** -0.5
N_WARM = 24       # HAM warm-up matmuls (bridges to the c0 arrival)


def build_nc():
    nc = bacc.Bacc("TRN2", target_bir_lowering=False, debug=False)

    xT_d = nc.dram_tensor("xT", [D, SEQ], AT, kind="ExternalInput")
    cT_d = nc.dram_tensor("cT", [D, SEQ], AT, kind="ExternalInput")
    # host-packed: [128, 3*NK*128]; block (w, k) holds W_w[k*128:(k+1)*128, :]
    # with the chunk's rows on the partition axis.
    wqkv_d = nc.dram_tensor("wqkv", [128, 3 * NK * 128], AT, kind="ExternalInput")
    wo_d = nc.dram_tensor("wo", [F, D], BF16, kind="ExternalInput")
    out_d = nc.dram_tensor("out_p", [SEQ, D], F16, kind="ExternalOutput")

    with tile.TileContext(nc) as tc:
        _emit(tc, nc, xT_d, cT_d, wqkv_d, wo_d, out_d)
    nc.compile()
    return nc


def _emit(tc, nc, xT_d, cT_d, wqkv_d, wo_d, out_d):
    from contextlib import ExitStack

    ctx = ExitStack()
    wpool = ctx.enter_context(tc.tile_pool(name="wpool", bufs=1))
    big = ctx.enter_context(tc.tile_pool(name="big", bufs=1))
    ptp = ctx.enter_context(tc.tile_pool(name="ptp", bufs=6))
    ostage = ctx.enter_context(tc.tile_pool(name="ostage", bufs=2))
    # PSUM budget (8 banks x 2KB):
    #   st ring  : 2 x [128,1024] f32 = 4 banks
    #   oaug ring: 2 x [65,512]  f32 = 2 banks
    #   acc ring : 2 x [128,512] f32 = 2 banks (kacc/vacc/qacc/tp/bc/ops)
    ps_st = ctx.enter_context(tc.tile_pool(name="ps_st", bufs=2, space="PSUM"))
    ps_acc = ctx.enter_context(tc.tile_pool(name="ps_acc", bufs=2, space="PSUM"))
    ps_oaug = ctx.enter_context(tc.tile_pool(name="ps_oaug", bufs=2, space="PSUM"))

    # ---- constants ----
    ident = wpool.tile([128, 128], BF16, name="ident")
    make_identity(nc, ident)
    zbias = wpool.tile([128, 1], F32, name="zbias")
    nc.vector.memset(zbias, 0.0)
    ones64 = wpool.tile([1, DH], BF16, name="ones64")
    nc.vector.memset(ones64, 1.0)
    warm = wpool.tile([128, 512], BF16, name="warm")
    nc.vector.memset(warm, 0.0)

    # ---- input DMAs: one sync-ring queue, program order = arrival order ----
    wqkv_s = wpool.tile([128, 3, NK, 128], AT, name="wqkv_s")
    nc.scalar.dma_start(out=wqkv_s.rearrange("p a b c -> p (a b c)"), in_=wqkv_d.ap())
    cS = wpool.tile([128, NK, SEQ], AT, name="cS")
    for k in range(2):  # first chunks small so the k/v chase starts early
        nc.sync.dma_start(
            out=cS[:, k, :], in_=cT_d.ap()[k * 128 : (k + 1) * 128, :]
        )
    for k in range(2, NK, 2):
        nc.sync.dma_start(
            out=cS[:, k : k + 2, :],
            in_=cT_d.ap()[k * 128 : (k + 2) * 128, :].rearrange(
                "(j p) c -> p j c", j=2
            ),
        )
    xS = wpool.tile([128, NK, SEQ], AT, name="xS")
    for k in range(0, NK, 2):
        nc.sync.dma_start(
            out=xS[:, k : k + 2, :],
            in_=xT_d.ap()[k * 128 : (k + 2) * 128, :].rearrange(
                "(j p) c -> p j c", j=2
            ),
        )
    wo_s = wpool.tile([128, D], BF16, name="wo_s")
    nc.sync.dma_start(out=wo_s, in_=wo_d.ap())

    # ---- persistent activations ----
    qT = big.tile([128, SEQ], BF16, name="qT", tag="qT")
    kT = big.tile([128, SEQ], BF16, name="kT", tag="kT")
    vT = big.tile([128, SEQ], BF16, name="vT", tag="vT")
    OT = big.tile([128, SEQ], BF16, name="OT", tag="OT")
    # V per head+m-chunk, with a ones column (65th) that accumulates the
    # softmax denominators during the PV matmul.
    Vall = big.tile([128, 2, NM, VPAD], BF16, name="Vall", tag="Vall")
    ones_sb = wpool.tile([128, 2 * NM], F32, name="ones_sb")
    nc.vector.memset(ones_sb, 1.0)
    nc.vector.tensor_copy(
        out=Vall[:, :, :, DH : DH + 1],
        in_=ones_sb.rearrange("p (h m o) -> p h m o", h=2, o=1),
    )
    zpad_sb = wpool.tile([128, 2 * NM * (VPAD - DH - 1)], F32, name="zpad_sb")
    nc.vector.memset(zpad_sb, 0.0)
    nc.vector.tensor_copy(
        out=Vall[:, :, :, DH + 1 :],
        in_=zpad_sb.rearrange("p (h m o) -> p h m o", h=2, m=NM),
    )

    # ---- HAM warm-up: >=3.4us of sustained PE streaming with no DMA deps.
    # Row-group alternation lets each LDWEIGHTS overlap the other group's
    # in-flight matmul, so the PE array streams continuously.
    wps = ps_st.tile([128, 1024], F32, name="wps", tag="st")
    for i in range(N_WARM):
        h = i % 2
        nc.tensor.matmul(
            wps[:, 512 * h : 512 * (h + 1)],
            warm[h * 64 : (h + 1) * 64, 0:128],
            warm[h * 64 : (h + 1) * 64, 0:512],
            start=True, stop=True, tile_position=(h * 64, 0),
        )

    # ---- projections ----
    # q for ALL 4 n-chunks chases the xS DMAs with 4 live accumulators
    # (acc ring + oaug ring; kv then reuses those slots).  k/v keeps all
    # 8 accumulators live so the cS chunks are visited in arrival order.
    def qproj_all():
        qa = ps_acc.tile([128, 512], F32, name="qa", tag="acc")
        qb = ps_acc.tile([128, 512], F32, name="qb", tag="acc")
        qc = ps_oaug.tile([128, 512], F32, name="qc", tag="oaug")
        qd = ps_oaug.tile([128, 512], F32, name="qd", tag="oaug")
        accs = [qa, qb, qc, qd]
        for k in range(NK):
            for s in range(4):
                nc.tensor.matmul(
                    accs[s], wqkv_s[:, 0, k, :], xS[:, k, s * 512 : (s + 1) * 512],
                    start=(k == 0), stop=(k == NK - 1),
                )
        for s in range(4):
            eng = nc.vector.tensor_copy if s % 2 == 0 else nc.scalar.copy
            eng(out=qT[:, s * 512 : (s + 1) * 512], in_=accs[s])

    def kvproj_all():
        """k/v for all m in one k-major chase over the cS chunks; the
        evacuations run g=3 first (frees the oaug ring for mk_oaug(0)
        before the attention PV stream reaches the PE queue head)."""
        k0 = ps_acc.tile([128, 512], F32, name="k0", tag="acc")
        v0 = ps_acc.tile([128, 512], F32, name="v0", tag="acc")
        kv1 = ps_st.tile([128, 1024], F32, name="kv1", tag="st")
        kv2 = ps_st.tile([128, 1024], F32, name="kv2", tag="st")
        k3 = ps_oaug.tile([128, 512], F32, name="k3", tag="oaug")
        v3 = ps_oaug.tile([128, 512], F32, name="v3", tag="oaug")
        kaccs = [k0, kv1[:, 0:512], kv2[:, 0:512], k3]
        vaccs = [v0, kv1[:, 512:1024], kv2[:, 512:1024], v3]
        for k in range(NK):
            for g in range(4):
                nc.tensor.matmul(
                    kaccs[g], wqkv_s[:, 1, k, :], cS[:, k, g * 512 : (g + 1) * 512],
                    start=(k == 0), stop=(k == NK - 1),
                )
            for g in range(4):
                nc.tensor.matmul(
                    vaccs[g], wqkv_s[:, 2, k, :], cS[:, k, g * 512 : (g + 1) * 512],
                    start=(k == 0), stop=(k == NK - 1),
                )
        for g in (3, 0, 1, 2):
            nc.vector.tensor_copy(out=kT[:, g * 512 : (g + 1) * 512], in_=kaccs[g])
            nc.scalar.copy(out=vT[:, g * 512 : (g + 1) * 512], in_=vaccs[g])

    def vtrans(g):
        """Vall[:, h, mc, 0:64] = vT[h*64:(h+1)*64, mc*128:(mc+1)*128].T
        Both heads in one [128,128] PE transpose."""
        for mc in range(4 * g, 4 * g + 4):
            tp = ps_acc.tile([128, 128], BF16, name="tp", tag="acc")
            nc.tensor.transpose(tp, vT[:, mc * 128 : (mc + 1) * 128], ident)
            nc.vector.tensor_copy(
                out=Vall[:, :, mc, 0:DH],
                in_=tp.rearrange("p (h d) -> p h d", h=2),
            )

    def attn_one(s, oaug, mc):
        """One m-chunk of attention for n-chunk s."""
        n0, n1 = s * 512, (s + 1) * 512
        m0, m1 = mc * 128, (mc + 1) * 128
        st = ps_st.tile([128, 1024], F32, name="st", tag="st")
        nc.tensor.matmul(
            st[:, 0:512], kT[0:DH, m0:m1], qT[0:DH, n0:n1],
            start=True, stop=True, tile_position=(0, 0),
        )
        nc.tensor.matmul(
            st[:, 512:1024], kT[DH:128, m0:m1], qT[DH:128, n0:n1],
            start=True, stop=True, tile_position=(64, 0),
        )
        pt = ptp.tile([128, 1024], BF16, name="pt", tag="pt")
        nc.scalar.activation(
            out=pt, in_=st,
            func=mybir.ActivationFunctionType.Exp,
            bias=zbias, scale=SCALE,
        )
        nc.tensor.matmul(
            oaug[0], Vall[:, 0, mc, 0 : DH + 1], pt[:, 0:512],
            start=(mc == 0), stop=(mc == NM - 1),
        )
        nc.tensor.matmul(
            oaug[1], Vall[:, 1, mc, 0 : DH + 1], pt[:, 512:1024],
            start=(mc == 0), stop=(mc == NM - 1),
        )

    fin_state = {}

    pt_live = {}

    def emit_st(s, mc):
        """Score matmuls + exp for one (s, mc) slot."""
        n0, n1 = s * 512, (s + 1) * 512
        m0, m1 = mc * 128, (mc + 1) * 128
        st = ps_st.tile([128, 1024], F32, name="st", tag="st")
        nc.tensor.matmul(
            st[:, 0:512], kT[0:DH, m0:m1], qT[0:DH, n0:n1],
            start=True, stop=True, tile_position=(0, 0),
        )
        nc.tensor.matmul(
            st[:, 512:1024], kT[DH:128, m0:m1], qT[DH:128, n0:n1],
            start=True, stop=True, tile_position=(64, 0),
        )
        pt = ptp.tile([128, 1024], BF16, name="pt", tag="pt")
        nc.scalar.activation(
            out=pt, in_=st,
            func=mybir.ActivationFunctionType.Exp,
            bias=zbias, scale=SCALE,
        )
        pt_live[(s, mc)] = pt

    def emit_pv(s, oaug, mc):
        pt = pt_live.pop((s, mc))
        nc.tensor.matmul(
            oaug[0], Vall[:, 0, mc, 0 : DH + 1], pt[:, 0:512],
            start=(mc == 0), stop=(mc == NM - 1),
        )
        nc.tensor.matmul(
            oaug[1], Vall[:, 1, mc, 0 : DH + 1], pt[:, 512:1024],
            start=(mc == 0), stop=(mc == NM - 1),
        )

    def attn_pipe2(s, oaug):
        """First two m-chunks of a phase with both St/exp pairs issued
        ahead of the PVs, so the PE queue head doesn't block on
        fin_pre's oaug evacuations (PV mc0 start=True waits on them)."""
        n0, n1 = s * 512, (s + 1) * 512
        pts = []
        for mc in (0, 1):
            m0, m1 = mc * 128, (mc + 1) * 128
            st = ps_st.tile([128, 1024], F32, name="st", tag="st")
            nc.tensor.matmul(
                st[:, 0:512], kT[0:DH, m0:m1], qT[0:DH, n0:n1],
                start=True, stop=True, tile_position=(0, 0),
            )
            nc.tensor.matmul(
                st[:, 512:1024], kT[DH:128, m0:m1], qT[DH:128, n0:n1],
                start=True, stop=True, tile_position=(64, 0),
            )
            pt = ptp.tile([128, 1024], BF16, name="pt", tag="pt")
            nc.scalar.activation(
                out=pt, in_=st,
                func=mybir.ActivationFunctionType.Exp,
                bias=zbias, scale=SCALE,
            )
            pts.append(pt)
        for mc in (0, 1):
            nc.tensor.matmul(
                oaug[0], Vall[:, 0, mc, 0 : DH + 1], pts[mc][:, 0:512],
                start=(mc == 0), stop=False,
            )
            nc.tensor.matmul(
                oaug[1], Vall[:, 1, mc, 0 : DH + 1], pts[mc][:, 512:1024],
                start=(mc == 0), stop=False,
            )

    def fin_pre(s, oaug, last=False):
        """Start softmax-denominator normalization for n-chunk s: evacuate
        oaug, repartition the [1,512] denominator row to [128,4]
        (SBUF->SBUF DMA), reciprocal on all DVE lanes, DMA back to a
        bf16 [1,512] row.  All DMAs for both heads are interleaved so
        the two chains pipeline.  No PE work here -- the bc broadcast
        matmuls are emitted later (fin_bc) so the PE queue head never
        blocks on this chain's DMA latency."""
        sbs, recs = [], []
        for h in range(2):
            oaug_sb = ostage.tile([DH + 1, 512], F32, name="oaug_sb",
                                  tag="oaug_sb", bufs=2)
            if last and h == 0:
                nc.scalar.copy(out=oaug_sb, in_=oaug[h])
            else:
                nc.vector.tensor_copy(out=oaug_sb, in_=oaug[h])
            sbs.append(oaug_sb)
        dens = []
        for h in range(2):
            den_p = ostage.tile([128, 4], F32, name="den_p", tag="den_p", bufs=2)
            nc.sync.dma_start(out=den_p, in_=sbs[h][DH : DH + 1, :])
            dens.append(den_p)
        rps = []
        for h in range(2):
            rec_p = ostage.tile([128, 4], BF16, name="rec_p", tag="rec_p", bufs=2)
            with nc.allow_low_precision(reason="bf16 softmax denominators"):
                nc.vector.reciprocal(out=rec_p, in_=dens[h])
            rps.append(rec_p)
        for h in range(2):
            rec_row = ostage.tile([1, 512], BF16, name="rec_row", tag="rec_row",
                                  bufs=2)
            nc.sync.dma_start(out=rec_row, in_=rps[h])
            recs.append(rec_row)
        fin_state[s] = (sbs, recs)

    def fin_bc(s):
        """Finish fin: broadcast the reciprocal row to 64 partitions with
        a bf16 PE matmul and scale oaug into OT."""
        n0, n1 = s * 512, (s + 1) * 512
        sbs, recs = fin_state.pop(s)
        for h in range(2):
            bc = ps_acc.tile([DH, 512], F32, name="bc", tag="acc")
            nc.tensor.matmul(bc, ones64, recs[h], start=True, stop=True)
            nc.vector.tensor_mul(
                out=OT[h * DH : (h + 1) * DH, n0:n1],
                in0=sbs[h][0:DH, :],
                in1=bc,
            )

    def outproj_piece(s, i, tail=False):
        """Piece i (of 8) of the out-projection for n-chunk s.  The bias
        bo is added on the host during the partial-sum gather."""
        nt = s * 4 + i // 2
        half = i % 2
        c0, c1 = half * 512, (half + 1) * 512
        ops = ps_acc.tile([128, 512], F32, name="ops", tag="acc")
        nc.tensor.matmul(
            ops, OT[:, nt * 128 : (nt + 1) * 128], wo_s[:, c0:c1],
            start=True, stop=True,
        )
        osb = ostage.tile([128, 512], F16, name="osb", tag="osb", bufs=4)
        with nc.allow_low_precision(reason="f16 partial outputs"):
            if tail and i % 2 == 1:
                nc.scalar.copy(out=osb, in_=ops)
            else:
                nc.vector.tensor_copy(out=osb, in_=ops)
        if tail:
            eng = nc.sync if i % 2 == 0 else nc.gpsimd
        else:
            eng = nc.sync if i % 2 == 1 else nc.gpsimd
        eng.dma_start(out=out_d.ap()[nt * 128 : (nt + 1) * 128, c0:c1], in_=osb)

    # ---- schedule ----
    def mk_oaug(s):
        return [
            ps_oaug.tile([DH + 1, 512], F32, name=f"oaug{s}_{h}", tag="oaug")
            for h in range(2)
        ]

    wf_tile = []

    def warm_fill(n):
        """Dummy matmuls into a free st-ring slot to keep/restore HAM."""
        if not wf_tile:
            wf_tile.append(ps_st.tile([128, 1024], F32, name="wf", tag="st"))
        wf = wf_tile[0]
        for i in range(n):
            h = i % 2
            nc.tensor.matmul(
                wf[:, 512 * h : 512 * (h + 1)],
                warm[h * 64 : (h + 1) * 64, 0:128],
                warm[h * 64 : (h + 1) * 64, 0:512],
                start=True, stop=True, tile_position=(h * 64, 0),
            )

    kvproj_all()
    qproj_all()
    vtrans(0)
    oaugs = {0: mk_oaug(0)}
    slots = [(s, mc) for s in range(NS) for mc in range(NM)]
    # St/exp for slot i+2 issues right after PV of slot i, so the score
    # matmuls for the next phase run during the current phase's last
    # exps and the exp stream never pauses at phase boundaries.
    emit_st(*slots[0])
    emit_st(*slots[1])
    for i, (s, mc) in enumerate(slots):
        if s == 0 and mc % 4 == 0 and mc < 12:
            vtrans(mc // 4 + 1)
        if s >= 1 and mc == 0:
            fin_pre(s - 1, oaugs[s - 1])
            oaugs[s] = mk_oaug(s)
        if s >= 1 and mc == 6:
            fin_bc(s - 1)
        if s >= 1 and 7 <= mc <= 14:
            outproj_piece(s - 1, mc - 7)
        emit_pv(s, oaugs[s], mc)
        if i + 2 < len(slots):
            emit_st(*slots[i + 2])
    oaug_cur = oaugs[NS - 1]
    fin_pre(NS - 1, oaug_cur, last=True)
    warm_fill(48)
    fin_bc(NS - 1)
    for i in range(8):
        outproj_piece(NS - 1, i, tail=True)
        if i < 7:
            warm_fill(2)

    ctx.close()


_NC = None


def _get_nc():
    global _NC
    if _NC is None:
        _NC = build_nc()
    return _NC


def _np_at():
    import ml_dtypes

    return ml_dtypes.float8_e4m3 if USE_FP8_INPUTS else ml_dtypes.bfloat16


def _swizzle(w):
    """[1024, 128] -> [128, 8*128]: chunk k of the contraction dim lands in
    column block k with the chunk's rows on the partition axis."""
    return (
        np.asarray(w, np.float32).reshape(NK, 128, F).transpose(1, 0, 2)
        .reshape(128, NK * F)
    )


def shard_inputs(x, context, Wq, Wk, Wv, Wo, bo):
    import ml_dtypes

    x = np.asarray(x, np.float32)
    context = np.asarray(context, np.float32)
    Wq = np.asarray(Wq, np.float32)
    Wk = np.asarray(Wk, np.float32)
    Wv = np.asarray(Wv, np.float32)
    Wo = np.asarray(Wo, np.float32)
    bo = np.asarray(bo, np.float32)

    at = _np_at()
    xT = [np.ascontiguousarray(x[b].T).astype(at) for b in range(x.shape[0])]
    cT = [np.ascontiguousarray(context[b].T).astype(at) for b in range(context.shape[0])]
    in_maps = []
    for c in range(8):
        b, hp = divmod(c, 4)
        f0 = hp * F
        wqkv = np.ascontiguousarray(
            np.concatenate(
                [
                    _swizzle(Wq[:, f0 : f0 + F]),
                    _swizzle(Wk[:, f0 : f0 + F]),
                    _swizzle(Wv[:, f0 : f0 + F]),
                ],
                axis=1,
            )
        ).astype(at)
        in_maps.append(
            {
                "xT": xT[b],
                "cT": cT[b],
                "wqkv": wqkv,
                "wo": np.ascontiguousarray(Wo[f0 : f0 + F, :]).astype(
                    ml_dtypes.bfloat16
                ),
            }
        )
    return in_maps


def kernel(x, context, Wq, Wk, Wv, Wo, bo):
    from concourse.bass_utils import run_bass_kernel_spmd

    in_maps = shard_inputs(x, context, Wq, Wk, Wv, Wo, bo)
    nc = _get_nc()
    res = run_bass_kernel_spmd(nc, in_maps, list(range(8)))
    out = np.zeros((2, SEQ, D), np.float32)
    for c in range(8):
        out[c // 4] += np.asarray(res.results[c]["out_p"], np.float32)
    out += np.asarray(bo, np.float32)  # bias folded into the gather
    return out


# revision 26
# speedup vs baseline: 1.0569x; 1.0038x over previous
"""Cross-attention kernel for Trainium2, 8 NeuronCores.

Sharding (data + head parallel, per the problem's sharding hint):
  core c in 0..7 -> batch b = c // 4, head-pair hp = c % 4.
  Each core computes attention for its batch with 2 of the 8 heads
  (a 128-wide slice of the 512 hidden features), then the partial
  out-projection  attn_out_slice @ Wo[slice, :].  The host sums the 4
  partials per batch (the "all-reduce"); bo is added on the hp==0 core.

Performance structure:
  - Inputs land in SBUF via 17 LARGE DMAs on the sync HWDGE ring
    (per-dma_start issue cost is ~0.6us on the issuing engine, so small
    DMAs serialize); x before context.
  - fp8e4m3 input option halves the input-DMA wall that gates the first
    attention matmul (q needs ALL of x, k needs ALL of context).
  - ~4us of row-group-alternating dummy matmuls at t=0 warm the PE HAM
    clock gate (cold PE = 1.2 GHz, warm = 2.4 GHz).
  - k/v projections keep 8 PSUM accumulators live (acc ring + st-slot
    halves + oaug ring) so the contraction chunks are visited in
    DMA-arrival order.
  - The body is ScalarE-bound (64 exp ACTIVATEs of [128,1024] at
    ~1.1us each): all other PE work (qproj of the NEXT n-chunk,
    outproj of the PREVIOUS one) is interleaved one matmul per
    attention step so the exp stream never pauses.
  - fin() is all-bf16: SBUF->SBUF repartition DMAs + a bf16 PE
    broadcast matmul (an fp32 matmul lowers to two ~1us passes!).
  - Partial outputs are written as f16 (halves output DMA); the host
    sums the 4 partials per batch in f32.
"""

import numpy as np

import concourse.bass as bass
import concourse.tile as tile
from concourse import bacc, mybir
from concourse.masks import make_identity

F32 = mybir.dt.float32
F16 = mybir.dt.float16
BF16 = mybir.dt.bfloat16
F8 = mybir.dt.float8e4

USE_FP8_INPUTS = False   # fp8 x/context fails accuracy: dot-product error does not average down
AT = F8 if USE_FP8_INPUTS else BF16
VPAD = 72                # PV weight row padded to 16B-aligned stride (bf16)

D = 1024      # model dim (contraction for projections)
SEQ = 2048    # n == m
F = 128       # features per core (2 heads x 64)
DH = 64       # head dim
NS = SEQ // 512   # 4 n-chunks of 512
NK = D // 128     # 8 contraction chunks
NM = SEQ // 128   # 16 m-chunks of 128
SCALE = DH ** -0.5
N_WARM = 16       # HAM warm-up matmuls (bridges to the c0 arrival)


def build_nc():
    nc = bacc.Bacc("TRN2", target_bir_lowering=False, debug=False)

    xT_d = nc.dram_tensor("xT", [D, SEQ], AT, kind="ExternalInput")
    cT_d = nc.dram_tensor("cT", [D, SEQ], AT, kind="ExternalInput")
    # host-packed: [128, 3*NK*128]; block (w, k) holds W_w[k*128:(k+1)*128, :]
    # with the chunk's rows on the partition axis.
    wqkv_d = nc.dram_tensor("wqkv", [128, 3 * NK * 128], AT, kind="ExternalInput")
    wo_d = nc.dram_tensor("wo", [F, D], BF16, kind="ExternalInput")
    out_d = nc.dram_tensor("out_p", [SEQ, D], F16, kind="ExternalOutput")

    with tile.TileContext(nc) as tc:
        _emit(tc, nc, xT_d, cT_d, wqkv_d, wo_d, out_d)
    nc.compile()
    return nc


def _emit(tc, nc, xT_d, cT_d, wqkv_d, wo_d, out_d):
    from contextlib import ExitStack

    ctx = ExitStack()
    wpool = ctx.enter_context(tc.tile_pool(name="wpool", bufs=1))
    big = ctx.enter_context(tc.tile_pool(name="big", bufs=1))
    ptp = ctx.enter_context(tc.tile_pool(name="ptp", bufs=6))
    ostage = ctx.enter_context(tc.tile_pool(name="ostage", bufs=2))
    # PSUM budget (8 banks x 2KB):
    #   st ring  : 2 x [128,1024] f32 = 4 banks
    #   oaug ring: 2 x [65,512]  f32 = 2 banks
    #   acc ring : 2 x [128,512] f32 = 2 banks (kacc/vacc/qacc/tp/bc/ops)
    ps_st = ctx.enter_context(tc.tile_pool(name="ps_st", bufs=2, space="PSUM"))
    ps_acc = ctx.enter_context(tc.tile_pool(name="ps_acc", bufs=2, space="PSUM"))
    ps_oaug = ctx.enter_context(tc.tile_pool(name="ps_oaug", bufs=2, space="PSUM"))

    # ---- constants ----
    ident = wpool.tile([128, 128], BF16, name="ident")
    make_identity(nc, ident)
    zbias = wpool.tile([128, 1], F32, name="zbias")
    nc.vector.memset(zbias, 0.0)
    ones64 = wpool.tile([1, DH], BF16, name="ones64")
    nc.vector.memset(ones64, 1.0)
    warm = wpool.tile([128, 512], BF16, name="warm")
    nc.vector.memset(warm, 0.0)

    # ---- input DMAs: one sync-ring queue, program order = arrival order ----
    wqkv_s = wpool.tile([128, 3, NK, 128], AT, name="wqkv_s")
    nc.scalar.dma_start(out=wqkv_s.rearrange("p a b c -> p (a b c)"), in_=wqkv_d.ap())
    cS = wpool.tile([128, NK, SEQ], AT, name="cS")
    for k in range(2):  # first chunks small so the k/v chase starts early
        nc.sync.dma_start(
            out=cS[:, k, :], in_=cT_d.ap()[k * 128 : (k + 1) * 128, :]
        )
    for k in range(2, NK, 2):
        nc.sync.dma_start(
            out=cS[:, k : k + 2, :],
            in_=cT_d.ap()[k * 128 : (k + 2) * 128, :].rearrange(
                "(j p) c -> p j c", j=2
            ),
        )
    xS = wpool.tile([128, NK, SEQ], AT, name="xS")
    for k in range(0, NK, 2):
        nc.sync.dma_start(
            out=xS[:, k : k + 2, :],
            in_=xT_d.ap()[k * 128 : (k + 2) * 128, :].rearrange(
                "(j p) c -> p j c", j=2
            ),
        )
    wo_s = wpool.tile([128, D], BF16, name="wo_s")
    nc.sync.dma_start(out=wo_s, in_=wo_d.ap())

    # ---- persistent activations ----
    qT = big.tile([128, SEQ], BF16, name="qT", tag="qT")
    kT = big.tile([128, SEQ], BF16, name="kT", tag="kT")
    vT = big.tile([128, SEQ], BF16, name="vT", tag="vT")
    OT = big.tile([128, SEQ], BF16, name="OT", tag="OT")
    # V per head+m-chunk, with a ones column (65th) that accumulates the
    # softmax denominators during the PV matmul.
    Vall = big.tile([128, 2, NM, VPAD], BF16, name="Vall", tag="Vall")
    ones_sb = wpool.tile([128, 2 * NM], F32, name="ones_sb")
    nc.vector.memset(ones_sb, 1.0)
    nc.vector.tensor_copy(
        out=Vall[:, :, :, DH : DH + 1],
        in_=ones_sb.rearrange("p (h m o) -> p h m o", h=2, o=1),
    )
    zpad_sb = wpool.tile([128, 2 * NM * (VPAD - DH - 1)], F32, name="zpad_sb")
    nc.vector.memset(zpad_sb, 0.0)
    nc.vector.tensor_copy(
        out=Vall[:, :, :, DH + 1 :],
        in_=zpad_sb.rearrange("p (h m o) -> p h m o", h=2, m=NM),
    )

    # ---- HAM warm-up: >=3.4us of sustained PE streaming with no DMA deps.
    # Row-group alternation lets each LDWEIGHTS overlap the other group's
    # in-flight matmul, so the PE array streams continuously.
    wps = ps_st.tile([128, 1024], F32, name="wps", tag="st")
    for i in range(N_WARM):
        h = i % 2
        nc.tensor.matmul(
            wps[:, 512 * h : 512 * (h + 1)],
            warm[h * 64 : (h + 1) * 64, 0:128],
            warm[h * 64 : (h + 1) * 64, 0:512],
            start=True, stop=True, tile_position=(h * 64, 0),
        )

    # ---- projections ----
    # q for ALL 4 n-chunks chases the xS DMAs with 4 live accumulators
    # (acc ring + oaug ring; kv then reuses those slots).  k/v keeps all
    # 8 accumulators live so the cS chunks are visited in arrival order.
    def qproj_all():
        qa = ps_acc.tile([128, 512], F32, name="qa", tag="acc")
        qb = ps_acc.tile([128, 512], F32, name="qb", tag="acc")
        qc = ps_oaug.tile([128, 512], F32, name="qc", tag="oaug")
        qd = ps_oaug.tile([128, 512], F32, name="qd", tag="oaug")
        accs = [qa, qb, qc, qd]
        for k in range(NK):
            for s in range(4):
                nc.tensor.matmul(
                    accs[s], wqkv_s[:, 0, k, :], xS[:, k, s * 512 : (s + 1) * 512],
                    start=(k == 0), stop=(k == NK - 1),
                )
        for s in range(4):
            eng = nc.vector.tensor_copy if s % 2 == 0 else nc.scalar.copy
            eng(out=qT[:, s * 512 : (s + 1) * 512], in_=accs[s])

    def kvproj_all():
        """k/v for all m in one k-major chase over the cS chunks; the
        evacuations run g=3 first (frees the oaug ring for mk_oaug(0)
        before the attention PV stream reaches the PE queue head)."""
        k0 = ps_acc.tile([128, 512], F32, name="k0", tag="acc")
        v0 = ps_acc.tile([128, 512], F32, name="v0", tag="acc")
        kv1 = ps_st.tile([128, 1024], F32, name="kv1", tag="st")
        kv2 = ps_st.tile([128, 1024], F32, name="kv2", tag="st")
        k3 = ps_oaug.tile([128, 512], F32, name="k3", tag="oaug")
        v3 = ps_oaug.tile([128, 512], F32, name="v3", tag="oaug")
        kaccs = [k0, kv1[:, 0:512], kv2[:, 0:512], k3]
        vaccs = [v0, kv1[:, 512:1024], kv2[:, 512:1024], v3]
        for k in range(NK):
            for g in range(4):
                nc.tensor.matmul(
                    kaccs[g], wqkv_s[:, 1, k, :], cS[:, k, g * 512 : (g + 1) * 512],
                    start=(k == 0), stop=(k == NK - 1),
                )
            for g in range(4):
                nc.tensor.matmul(
                    vaccs[g], wqkv_s[:, 2, k, :], cS[:, k, g * 512 : (g + 1) * 512],
                    start=(k == 0), stop=(k == NK - 1),
                )
        for g in (3, 0, 1, 2):
            nc.vector.tensor_copy(out=kT[:, g * 512 : (g + 1) * 512], in_=kaccs[g])
            nc.scalar.copy(out=vT[:, g * 512 : (g + 1) * 512], in_=vaccs[g])

    def vtrans(g):
        """Vall[:, h, mc, 0:64] = vT[h*64:(h+1)*64, mc*128:(mc+1)*128].T
        Both heads in one [128,128] PE transpose."""
        for mc in range(4 * g, 4 * g + 4):
            tp = ps_acc.tile([128, 128], BF16, name="tp", tag="acc")
            nc.tensor.transpose(tp, vT[:, mc * 128 : (mc + 1) * 128], ident)
            nc.vector.tensor_copy(
                out=Vall[:, :, mc, 0:DH],
                in_=tp.rearrange("p (h d) -> p h d", h=2),
            )

    def attn_one(s, oaug, mc):
        """One m-chunk of attention for n-chunk s."""
        n0, n1 = s * 512, (s + 1) * 512
        m0, m1 = mc * 128, (mc + 1) * 128
        st = ps_st.tile([128, 1024], F32, name="st", tag="st")
        nc.tensor.matmul(
            st[:, 0:512], kT[0:DH, m0:m1], qT[0:DH, n0:n1],
            start=True, stop=True, tile_position=(0, 0),
        )
        nc.tensor.matmul(
            st[:, 512:1024], kT[DH:128, m0:m1], qT[DH:128, n0:n1],
            start=True, stop=True, tile_position=(64, 0),
        )
        pt = ptp.tile([128, 1024], BF16, name="pt", tag="pt")
        nc.scalar.activation(
            out=pt, in_=st,
            func=mybir.ActivationFunctionType.Exp,
            bias=zbias, scale=SCALE,
        )
        nc.tensor.matmul(
            oaug[0], Vall[:, 0, mc, 0 : DH + 1], pt[:, 0:512],
            start=(mc == 0), stop=(mc == NM - 1),
        )
        nc.tensor.matmul(
            oaug[1], Vall[:, 1, mc, 0 : DH + 1], pt[:, 512:1024],
            start=(mc == 0), stop=(mc == NM - 1),
        )

    fin_state = {}

    pt_live = {}

    def emit_st(s, mc):
        """Score matmuls + exp for one (s, mc) slot."""
        n0, n1 = s * 512, (s + 1) * 512
        m0, m1 = mc * 128, (mc + 1) * 128
        st = ps_st.tile([128, 1024], F32, name="st", tag="st")
        nc.tensor.matmul(
            st[:, 0:512], kT[0:DH, m0:m1], qT[0:DH, n0:n1],
            start=True, stop=True, tile_position=(0, 0),
        )
        nc.tensor.matmul(
            st[:, 512:1024], kT[DH:128, m0:m1], qT[DH:128, n0:n1],
            start=True, stop=True, tile_position=(64, 0),
        )
        pt = ptp.tile([128, 1024], BF16, name="pt", tag="pt")
        nc.scalar.activation(
            out=pt, in_=st,
            func=mybir.ActivationFunctionType.Exp,
            bias=zbias, scale=SCALE,
        )
        pt_live[(s, mc)] = pt

    def emit_pv(s, oaug, mc):
        pt = pt_live.pop((s, mc))
        nc.tensor.matmul(
            oaug[0], Vall[:, 0, mc, 0 : DH + 1], pt[:, 0:512],
            start=(mc == 0), stop=(mc == NM - 1),
        )
        nc.tensor.matmul(
            oaug[1], Vall[:, 1, mc, 0 : DH + 1], pt[:, 512:1024],
            start=(mc == 0), stop=(mc == NM - 1),
        )

    def attn_pipe2(s, oaug):
        """First two m-chunks of a phase with both St/exp pairs issued
        ahead of the PVs, so the PE queue head doesn't block on
        fin_pre's oaug evacuations (PV mc0 start=True waits on them)."""
        n0, n1 = s * 512, (s + 1) * 512
        pts = []
        for mc in (0, 1):
            m0, m1 = mc * 128, (mc + 1) * 128
            st = ps_st.tile([128, 1024], F32, name="st", tag="st")
            nc.tensor.matmul(
                st[:, 0:512], kT[0:DH, m0:m1], qT[0:DH, n0:n1],
                start=True, stop=True, tile_position=(0, 0),
            )
            nc.tensor.matmul(
                st[:, 512:1024], kT[DH:128, m0:m1], qT[DH:128, n0:n1],
                start=True, stop=True, tile_position=(64, 0),
            )
            pt = ptp.tile([128, 1024], BF16, name="pt", tag="pt")
            nc.scalar.activation(
                out=pt, in_=st,
                func=mybir.ActivationFunctionType.Exp,
                bias=zbias, scale=SCALE,
            )
            pts.append(pt)
        for mc in (0, 1):
            nc.tensor.matmul(
                oaug[0], Vall[:, 0, mc, 0 : DH + 1], pts[mc][:, 0:512],
                start=(mc == 0), stop=False,
            )
            nc.tensor.matmul(
                oaug[1], Vall[:, 1, mc, 0 : DH + 1], pts[mc][:, 512:1024],
                start=(mc == 0), stop=False,
            )

    def fin_pre(s, oaug, last=False):
        """Start softmax-denominator normalization for n-chunk s: evacuate
        oaug, repartition the [1,512] denominator row to [128,4]
        (SBUF->SBUF DMA), reciprocal on all DVE lanes, DMA back to a
        bf16 [1,512] row.  All DMAs for both heads are interleaved so
        the two chains pipeline.  No PE work here -- the bc broadcast
        matmuls are emitted later (fin_bc) so the PE queue head never
        blocks on this chain's DMA latency."""
        sbs, recs = [], []
        for h in range(2):
            oaug_sb = ostage.tile([DH + 1, 512], F32, name="oaug_sb",
                                  tag="oaug_sb", bufs=2)
            if last and h == 0:
                nc.scalar.copy(out=oaug_sb, in_=oaug[h])
            else:
                nc.vector.tensor_copy(out=oaug_sb, in_=oaug[h])
            sbs.append(oaug_sb)
        dens = []
        for h in range(2):
            den_p = ostage.tile([128, 4], F32, name="den_p", tag="den_p", bufs=2)
            nc.sync.dma_start(out=den_p, in_=sbs[h][DH : DH + 1, :])
            dens.append(den_p)
        rps = []
        for h in range(2):
            rec_p = ostage.tile([128, 4], BF16, name="rec_p", tag="rec_p", bufs=2)
            with nc.allow_low_precision(reason="bf16 softmax denominators"):
                nc.vector.reciprocal(out=rec_p, in_=dens[h])
            rps.append(rec_p)
        for h in range(2):
            rec_row = ostage.tile([1, 512], BF16, name="rec_row", tag="rec_row",
                                  bufs=2)
            nc.sync.dma_start(out=rec_row, in_=rps[h])
            recs.append(rec_row)
        fin_state[s] = (sbs, recs)

    def fin_bc(s):
        """Finish fin: broadcast the reciprocal row to 64 partitions with
        a bf16 PE matmul and scale oaug into OT."""
        n0, n1 = s * 512, (s + 1) * 512
        sbs, recs = fin_state.pop(s)
        for h in range(2):
            bc = ps_acc.tile([DH, 512], F32, name="bc", tag="acc")
            nc.tensor.matmul(bc, ones64, recs[h], start=True, stop=True)
            nc.vector.tensor_mul(
                out=OT[h * DH : (h + 1) * DH, n0:n1],
                in0=sbs[h][0:DH, :],
                in1=bc,
            )

    def outproj_piece(s, i, tail=False):
        """Piece i (of 8) of the out-projection for n-chunk s.  The bias
        bo is added on the host during the partial-sum gather."""
        nt = s * 4 + i // 2
        half = i % 2
        c0, c1 = half * 512, (half + 1) * 512
        if tail and i % 2 == 1:
            ops = ps_oaug.tile([128, 512], F32, name="ops", tag="oaug")
        else:
            ops = ps_acc.tile([128, 512], F32, name="ops", tag="acc")
        nc.tensor.matmul(
            ops, OT[:, nt * 128 : (nt + 1) * 128], wo_s[:, c0:c1],
            start=True, stop=True,
        )
        osb = ostage.tile([128, 512], F16, name="osb", tag="osb", bufs=4)
        with nc.allow_low_precision(reason="f16 partial outputs"):
            if tail and i % 2 == 1:
                nc.scalar.copy(out=osb, in_=ops)
            else:
                nc.vector.tensor_copy(out=osb, in_=ops)
        if tail:
            eng = nc.sync if i % 2 == 0 else nc.gpsimd
        else:
            eng = nc.sync if i % 2 == 1 else nc.gpsimd
        eng.dma_start(out=out_d.ap()[nt * 128 : (nt + 1) * 128, c0:c1], in_=osb)

    # ---- schedule ----
    def mk_oaug(s):
        return [
            ps_oaug.tile([DH + 1, 512], F32, name=f"oaug{s}_{h}", tag="oaug")
            for h in range(2)
        ]

    wf_tile = []

    def warm_fill(n):
        """Dummy matmuls into a free st-ring slot to keep/restore HAM."""
        if not wf_tile:
            wf_tile.append(ps_st.tile([128, 1024], F32, name="wf", tag="st"))
        wf = wf_tile[0]
        for i in range(n):
            h = i % 2
            nc.tensor.matmul(
                wf[:, 512 * h : 512 * (h + 1)],
                warm[h * 64 : (h + 1) * 64, 0:128],
                warm[h * 64 : (h + 1) * 64, 0:512],
                start=True, stop=True, tile_position=(h * 64, 0),
            )

    kvproj_all()
    qproj_all()
    vtrans(0)
    oaugs = {0: mk_oaug(0)}
    slots = [(s, mc) for s in range(NS) for mc in range(NM)]
    # St/exp for slot i+2 issues right after PV of slot i, so the score
    # matmuls for the next phase run during the current phase's last
    # exps and the exp stream never pauses at phase boundaries.
    emit_st(*slots[0])
    emit_st(*slots[1])
    for i, (s, mc) in enumerate(slots):
        if s == 0 and mc % 4 == 0 and mc < 12:
            vtrans(mc // 4 + 1)
        if s >= 1 and mc == 0:
            fin_pre(s - 1, oaugs[s - 1])
            oaugs[s] = mk_oaug(s)
        if s >= 1 and mc == 6:
            fin_bc(s - 1)
        if s >= 1 and 7 <= mc <= 14:
            outproj_piece(s - 1, mc - 7)
        emit_pv(s, oaugs[s], mc)
        if i + 2 < len(slots):
            emit_st(*slots[i + 2])
    oaug_cur = oaugs[NS - 1]
    fin_pre(NS - 1, oaug_cur, last=True)
    warm_fill(48)
    fin_bc(NS - 1)
    for i in range(8):
        outproj_piece(NS - 1, i, tail=True)
        if i < 7:
            warm_fill(2)

    ctx.close()


_NC = None


def _get_nc():
    global _NC
    if _NC is None:
        _NC = build_nc()
    return _NC


def _np_at():
    import ml_dtypes

    return ml_dtypes.float8_e4m3 if USE_FP8_INPUTS else ml_dtypes.bfloat16


def _swizzle(w):
    """[1024, 128] -> [128, 8*128]: chunk k of the contraction dim lands in
    column block k with the chunk's rows on the partition axis."""
    return (
        np.asarray(w, np.float32).reshape(NK, 128, F).transpose(1, 0, 2)
        .reshape(128, NK * F)
    )


def shard_inputs(x, context, Wq, Wk, Wv, Wo, bo):
    import ml_dtypes

    x = np.asarray(x, np.float32)
    context = np.asarray(context, np.float32)
    Wq = np.asarray(Wq, np.float32)
    Wk = np.asarray(Wk, np.float32)
    Wv = np.asarray(Wv, np.float32)
    Wo = np.asarray(Wo, np.float32)
    bo = np.asarray(bo, np.float32)

    at = _np_at()
    xT = [np.ascontiguousarray(x[b].T).astype(at) for b in range(x.shape[0])]
    cT = [np.ascontiguousarray(context[b].T).astype(at) for b in range(context.shape[0])]
    in_maps = []
    for c in range(8):
        b, hp = divmod(c, 4)
        f0 = hp * F
        wqkv = np.ascontiguousarray(
            np.concatenate(
                [
                    _swizzle(Wq[:, f0 : f0 + F]),
                    _swizzle(Wk[:, f0 : f0 + F]),
                    _swizzle(Wv[:, f0 : f0 + F]),
                ],
                axis=1,
            )
        ).astype(at)
        in_maps.append(
            {
                "xT": xT[b],
                "cT": cT[b],
                "wqkv": wqkv,
                "wo": np.ascontiguousarray(Wo[f0 : f0 + F, :]).astype(
                    ml_dtypes.bfloat16
                ),
            }
        )
    return in_maps


def kernel(x, context, Wq, Wk, Wv, Wo, bo):
    from concourse.bass_utils import run_bass_kernel_spmd

    in_maps = shard_inputs(x, context, Wq, Wk, Wv, Wo, bo)
    nc = _get_nc()
    res = run_bass_kernel_spmd(nc, in_maps, list(range(8)))
    out = np.zeros((2, SEQ, D), np.float32)
    for c in range(8):
        out[c // 4] += np.asarray(res.results[c]["out_p"], np.float32)
    out += np.asarray(bo, np.float32)  # bias folded into the gather
    return out
